# revision 1
# baseline (speedup 1.0000x reference)
"""Grouped ESN (E=4, R=1024, B=16, T=1024, D=64) on 8 trn2 NeuronCores.

Sharding: 8 cores = 4 ESNs x 2 batch halves (B=8 per core). Each core runs the
full T=1024 sequential leaky-tanh recurrence for its (esn, batch-half):

    g_{t+1} = 0.7*g_t + tanh(W' @ g_t + u_t),   W' = 0.3*W[e],  g = h/0.3

Layouts (per core):
  - W' as stationary lhsT tiles [j_local, (it, kt, i_local)] bf16, 64 tiles of
    [128,128]; PE mapping: out[i,b] = sum_j W'[i,j] g[j,b], so out needs no
    transpose to become next step's rhs.
  - u[t] = W_in[e] @ x_t precomputed on device into SBUF as bf16
    [i_local, (it, t, b)], injected into PSUM via an identity matmul which also
    provides the start=True bank clear.
  - 4 PSUM banks (2 per step, ping-pong across steps); fp32 state master g32
    with ping-pong, bf16 rhs copy gbf with ping-pong.
"""

import sys
import numpy as np

sys.path.insert(0, "/opt/trn_rl_repo")

import ml_dtypes

E, D, R, B, T = 4, 64, 1024, 16, 1024
BC = B // 2          # batch per core
NIT = R // 128       # 8 i-tiles
NKT = R // 128       # 8 k-tiles
LEAK = 0.3
N_CORES = 8

_cached = {}


def _build_nc(n_steps):
    import concourse.bass as bass
    import concourse.mybir as mybir
    from concourse import bacc, tile

    fp32 = mybir.dt.float32
    bf16 = mybir.dt.float16  # fp16: finer mantissa than bf16, same PE speed/FWL
    AF = mybir.ActivationFunctionType
    ALU = mybir.AluOpType

    nc = bacc.Bacc("TRN2", target_bir_lowering=False, debug=False)

    wt_p = nc.declare_dram_parameter("wt", [128, NIT * NKT * 128], bf16, isOutput=False)
    wint_p = nc.declare_dram_parameter("wint", [D, R], bf16, isOutput=False)
    xt_p = nc.declare_dram_parameter("xt", [D, T * BC], bf16, isOutput=False)
    ident_p = nc.declare_dram_parameter("ident", [128, 128], bf16, isOutput=False)
    hout_p = nc.declare_dram_parameter("hout", [128, NIT * BC], fp32, isOutput=True)

    NCH = T * BC // 512  # 16 chunks of 512 (t,b) columns for u precompute

    with tile.TileContext(nc) as tc:
        with (
            tc.tile_pool(name="const", bufs=1) as cpool,
            tc.tile_pool(name="state", bufs=1) as spool,
            tc.tile_pool(name="ps1", bufs=4, space="PSUM") as ps1pool,
            tc.tile_pool(name="ps2", bufs=1, space="PSUM") as ps2pool,
        ):
            w_sb = cpool.tile([128, NIT * NKT * 128], bf16, tag="w")
            winT = cpool.tile([D, R], bf16, tag="winT")
            xt_sb = cpool.tile([D, T * BC], bf16, tag="xt")
            ident = cpool.tile([128, 128], bf16, tag="ident")
            u_sb = cpool.tile([128, NIT * T * BC], bf16, tag="u")

            g32 = [spool.tile([128, NIT * BC], fp32, tag=f"g32_{i}", name=f"g32_{i}") for i in range(2)]
            gbf = [spool.tile([128, NIT * BC], bf16, tag=f"gbf_{i}", name=f"gbf_{i}") for i in range(2)]
            th2 = [spool.tile([128, NIT * BC], fp32, tag=f"th_{i}", name=f"th_{i}") for i in range(2)]
            tmp2 = [spool.tile([128, NIT * BC], fp32, tag=f"tmp_{i}", name=f"tmp_{i}") for i in range(2)]

            pbank = [ps2pool.tile([128, 4 * BC], fp32, tag=f"pb_{i}", name=f"pb_{i}") for i in range(4)]

            nc.sync.dma_start(w_sb[:], wt_p[:])
            nc.sync.dma_start(winT[:], wint_p[:])
            nc.sync.dma_start(xt_sb[:], xt_p[:])
            nc.sync.dma_start(ident[:], ident_p[:])

            nc.vector.memset(g32[0][:], 0.0)
            nc.vector.memset(gbf[0][:], 0.0)

            # ---- Phase 1: u[i_local, (it, t, b)] = W_in[e] @ x_t, all t ----
            for c in range(NCH):
                for it in range(NIT):
                    pu = ps1pool.tile([128, 512], fp32)
                    nc.tensor.matmul(
                        pu[:],
                        winT[:, it * 128:(it + 1) * 128],
                        xt_sb[:, c * 512:(c + 1) * 512],
                        start=True, stop=True,
                    )
                    dst = u_sb[:, it * (T * BC) + c * 512: it * (T * BC) + (c + 1) * 512]
                    if it % 2 == 0:
                        nc.vector.tensor_copy(dst, pu[:])
                    else:
                        nc.scalar.activation(dst, pu[:], AF.Copy)

            u4 = u_sb[:].rearrange("p (i t b) -> p i t b", i=NIT, t=T, b=BC)

            # ---- Phase 2: scan ----
            for t in range(n_steps):
                gin = gbf[t % 2]
                gout = gbf[(t + 1) % 2]
                min_ = g32[t % 2]
                mout = g32[(t + 1) % 2]
                bx = pbank[(t % 2) * 2 + 0]   # i-tiles 0..3
                by = pbank[(t % 2) * 2 + 1]   # i-tiles 4..7

                # u inject (+ bank clear via start=True)
                nc.tensor.matmul(bx[:], ident[:], u4[:, 0:4, t, :],
                                 start=True, stop=False, skip_group_check=True)
                nc.tensor.matmul(by[:], ident[:], u4[:, 4:8, t, :],
                                 start=True, stop=False, skip_group_check=True)

                def wmm(itile, kt, bank, it0, last):
                    nc.tensor.matmul(
                        bank[:, (itile - it0) * BC:(itile - it0 + 1) * BC],
                        w_sb[:, (itile * NKT + kt) * 128:(itile * NKT + kt + 1) * 128],
                        gin[:, kt * BC:(kt + 1) * BC],
                        start=False, stop=last, skip_group_check=True,
                    )

                for it in range(0, 4):       # burst1: X, kt 0..3
                    for kt in range(0, 4):
                        wmm(it, kt, bx, 0, False)
                for it in range(0, 4):       # burst2: X, kt 4..7
                    for kt in range(4, 8):
                        wmm(it, kt, bx, 0, it == 3 and kt == 7)

                # post X (i-tiles 0..3 -> g columns 0..32)
                th = th2[t % 2]
                tmp = tmp2[t % 2]
                slx = slice(0, 4 * BC)
                nc.scalar.activation(th[:, slx], bx[:], AF.Tanh)
                nc.scalar.mul(tmp[:, slx], min_[:, slx], 1.0 - LEAK)
                nc.vector.tensor_add(mout[:, slx], tmp[:, slx], th[:, slx])
                nc.vector.tensor_copy(gout[:, slx], mout[:, slx])

                for it in range(4, 8):       # burst3: Y, kt 0..3
                    for kt in range(0, 4):
                        wmm(it, kt, by, 4, False)
                for it in range(4, 8):       # burst4: Y, kt 4..7
                    for kt in range(4, 8):
                        wmm(it, kt, by, 4, it == 7 and kt == 7)

                sly = slice(4 * BC, 8 * BC)
                nc.scalar.activation(th[:, sly], by[:], AF.Tanh)
                nc.scalar.mul(tmp[:, sly], min_[:, sly], 1.0 - LEAK)
                nc.vector.tensor_add(mout[:, sly], tmp[:, sly], th[:, sly])
                nc.vector.tensor_copy(gout[:, sly], mout[:, sly])

            nc.sync.dma_start(hout_p[:], g32[n_steps % 2][:])

    nc.compile()
    return nc


def _get_nc(n_steps=T):
    if n_steps not in _cached:
        _cached[n_steps] = _build_nc(n_steps)
    return _cached[n_steps]


def _prep_core_inputs(x, W_in, W, core):
    """Host-side layout prep for one core. x:[B,T,D] W_in:[E,R,D] W:[E,R,R]."""
    bf = np.float16
    e, bh = core // 2, core % 2
    bsl = slice(bh * BC, (bh + 1) * BC)

    # W' = 0.3*W[e]; lhsT layout [j_local, (it, kt, i_local)]
    Wp = (LEAK * W[e]).astype(np.float32)
    # [it, i_local, kt, j_local] -> [j_local, it, kt, i_local]
    w4 = Wp.reshape(NIT, 128, NKT, 128).transpose(3, 0, 2, 1).reshape(128, NIT * NKT * 128)

    winT = W_in[e].T.copy()               # [D, R]
    xt = x[bsl].transpose(2, 1, 0).reshape(D, T * BC)   # [d, (t, b)]

    return {
        "wt": np.ascontiguousarray(w4).astype(bf),
        "wint": np.ascontiguousarray(winT).astype(bf),
        "xt": np.ascontiguousarray(xt).astype(bf),
        "ident": np.eye(128, dtype=np.float32).astype(bf),
    }


def kernel(x, W_in, W):
    from concourse.bass_utils import run_bass_kernel_spmd

    x = np.asarray(x, dtype=np.float32)
    W_in = np.asarray(W_in, dtype=np.float32)
    W = np.asarray(W, dtype=np.float32)

    nc = _get_nc(T)
    in_maps = [_prep_core_inputs(x, W_in, W, c) for c in range(N_CORES)]
    res = run_bass_kernel_spmd(nc, in_maps, list(range(N_CORES))).results

    out = np.empty((B, E * R), dtype=np.float32)
    for c in range(N_CORES):
        e, bh = c // 2, c % 2
        g = np.asarray(res[c]["hout"], dtype=np.float32)   # [i_local, (it, b)]
        # h = 0.3*g ; out[bh*BC+b, e*R + it*128 + i_local]
        g3 = g.reshape(128, NIT, BC)
        out[bh * BC:(bh + 1) * BC, e * R:(e + 1) * R] = (
            LEAK * g3.transpose(2, 1, 0).reshape(BC, R)
        )
    return out


if __name__ == "__main__":
    sys.path.insert(0, "/root/problem")
    import reference

    ins = reference.setup_inputs()
    ins = {k: np.asarray(v) for k, v in ins.items()}
    exp = np.asarray(reference.reference(**ins))
    act = kernel(**ins)
    rel = np.linalg.norm(act - exp) / np.linalg.norm(exp)
    print("Relative error:", rel)

